# revision 18
# baseline (speedup 1.0000x reference)
"""Bounded attention (per-head QK RMSNorm + RoPE + KV-cache attention) on 8
Trainium2 NeuronCores.

Sharding: data parallel over batch. B=16 batches -> 2 per core; each core runs
all 16 heads over its own KV cache slice, no cross-core communication.

Per-core dataflow (all fp32):
  - Preprocess q,k (rmsnorm+rope) in a [128=(b,h,s), 128=d] layout, then one
    PE transpose each to get qT/kT_new in [d, (b,h,s)] layout.
  - Stream the KV cache in [128 rows x all-heads] row-groups (1 MiB contiguous
    DMAs), per head: PE-transpose k tile -> kT, mm1 sT[j,q] = kT.T @ qT,
    exp on ACT, mm2 oT[d,q] += v.T @ expT and sums[q] += ones.T @ expT,
    accumulated in a single PSUM bank for all 16 heads (one accumulation
    group: start on very first matmul, stop on the last).
  - Causal-masked 4x4 corner for the 4 new keys, then normalize by 1/sums and
    scatter to the output.
"""
import math
import numpy as np

import concourse.bass as bass
import concourse.tile as tile
from concourse import bacc, mybir
from concourse.bass_utils import run_bass_kernel_spmd

F32 = mybir.dt.float32
DEBUG = False
AF = mybir.ActivationFunctionType

B, S, DIM = 16, 4, 2048
H, D = 16, 128
KV = 4096
EPS = 1e-5
N_CORES = 8
B_LOC = B // N_CORES  # 2
TILES = KV // 128  # 32
SCALE = 1.0 / math.sqrt(D)
P = B_LOC * H * S  # 128 partitions in the (b, h, s) preproc layout


def _col(b, h):
    # column offset of (b, h)'s four queries in the qT/kT_new layouts
    return b * (H * S) + h * S


def _preprocess(nc, sb, pp, ps_pool, x_dram, w_sb, cos_sb, sin_sb, ident,
                eps_sb, name, dbg_x=None, dbg_xr=None):
    """rmsnorm + rope of q or k, returns transposed [d, (b,h,s)] SBUF tile."""
    # SBUF DMA APs must keep a single leading partition dim — load per (b, h)
    # so each transfer is [4, 128] at a plain partition base offset.
    x_sb = pp.tile([P, D], F32, tag=f"{name}_x")
    for b in range(B_LOC):
        for h in range(H):
            p0 = b * H * S + h * S
            nc.sync.dma_start(
                x_sb[p0:p0 + S, :], x_dram[b, :, h * D:(h + 1) * D]
            )
    sq = pp.tile([P, D], F32, tag="pp_sq")
    ssq = pp.tile([P, 1], F32, tag=f"{name}_ssq")
    nc.scalar.activation(sq[:], x_sb[:], AF.Square, accum_out=ssq[:])
    std = pp.tile([P, 1], F32, tag=f"{name}_std")
    nc.scalar.activation(std[:], ssq[:], AF.Sqrt, bias=eps_sb[:],
                         scale=1.0 / D)
    rinv = pp.tile([P, 1], F32, tag=f"{name}_rinv")
    nc.vector.reciprocal(rinv[:], std[:])
    xn = pp.tile([P, D], F32, tag=f"{name}_xn")
    nc.vector.tensor_scalar_mul(xn[:], x_sb[:], rinv[:])
    xnw = pp.tile([P, D], F32, tag=f"{name}_xnw")
    nc.vector.tensor_mul(xnw[:], xn[:], w_sb[:])

    # rope on even/odd interleaved pairs
    xv = xnw[:].rearrange("p (x two) -> p x two", two=2)
    a, bb = xv[:, :, 0], xv[:, :, 1]
    xr = pp.tile([P, D], F32, tag=f"{name}_xr")
    xrv = xr[:].rearrange("p (x two) -> p x two", two=2)
    t1 = pp.tile([P, D // 2], F32, tag="pp_t1")
    t2 = pp.tile([P, D // 2], F32, tag="pp_t2")
    nc.vector.tensor_mul(t1[:], a, cos_sb[:])
    nc.vector.tensor_mul(t2[:], bb, sin_sb[:])
    nc.vector.tensor_sub(xrv[:, :, 0], t1[:], t2[:])
    t3 = pp.tile([P, D // 2], F32, tag="pp_t1")
    t4 = pp.tile([P, D // 2], F32, tag="pp_t2")
    nc.vector.tensor_mul(t3[:], a, sin_sb[:])
    nc.vector.tensor_mul(t4[:], bb, cos_sb[:])
    nc.vector.tensor_add(xrv[:, :, 1], t3[:], t4[:])
    if dbg_x is not None:
        nc.sync.dma_start(dbg_x[:], x_sb[:])
        nc.sync.dma_start(dbg_xr[:], xr[:])

    # transpose -> [d, (b,h,s)]
    xT_ps = ps_pool.tile([D, P], F32, tag="kT_ps")
    nc.tensor.transpose(xT_ps[:], xr[:], ident[:])
    xT = sb.tile([D, P], F32, tag=f"{name}_T")
    nc.vector.tensor_copy(xT[:], xT_ps[:])
    return xT


def build():
    nc = bacc.Bacc("TRN2", target_bir_lowering=False, debug=False,
                   num_devices=N_CORES)

    q_d = nc.dram_tensor("q", [B_LOC, S, DIM], F32, kind="ExternalInput").ap()
    k_d = nc.dram_tensor("k", [B_LOC, S, DIM], F32, kind="ExternalInput").ap()
    v_d = nc.dram_tensor("v", [B_LOC, S, DIM], F32, kind="ExternalInput").ap()
    ck_d = nc.dram_tensor("cache_k", [B_LOC, KV, H, D], F32,
                          kind="ExternalInput").ap()
    cv_d = nc.dram_tensor("cache_v", [B_LOC, KV, H, D], F32,
                          kind="ExternalInput").ap()
    cos_d = nc.dram_tensor("cos_b", [P, D // 2], F32, kind="ExternalInput").ap()
    sin_d = nc.dram_tensor("sin_b", [P, D // 2], F32, kind="ExternalInput").ap()
    wq_d = nc.dram_tensor("wq_b", [P, D], F32, kind="ExternalInput").ap()
    wk_d = nc.dram_tensor("wk_b", [P, D], F32, kind="ExternalInput").ap()
    id_d = nc.dram_tensor("ident", [128, 128], F32, kind="ExternalInput").ap()
    ones_d = nc.dram_tensor("ones", [128, 1], F32, kind="ExternalInput").ap()
    mask_d = nc.dram_tensor("mask", [S, S], F32, kind="ExternalInput").ap()
    out_d = nc.dram_tensor("out", [B_LOC, S, DIM], F32,
                           kind="ExternalOutput").ap()
    if DEBUG:
        dbg_qT = nc.dram_tensor("dbg_qT", [128, 128], F32,
                                kind="ExternalOutput").ap()
        dbg_kTn = nc.dram_tensor("dbg_kTn", [128, 128], F32,
                                 kind="ExternalOutput").ap()
        dbg_acc = nc.dram_tensor("dbg_acc", [128, 256], F32,
                                 kind="ExternalOutput").ap()
        dbg_enm = nc.dram_tensor("dbg_enm", [H, S, S], F32,
                                 kind="ExternalOutput").ap()
        dbg_x = nc.dram_tensor("dbg_x", [128, 128], F32,
                               kind="ExternalOutput").ap()
        dbg_xr = nc.dram_tensor("dbg_xr", [128, 128], F32,
                                kind="ExternalOutput").ap()
    else:
        dbg_qT = dbg_kTn = dbg_acc = dbg_enm = dbg_x = dbg_xr = None

    with tile.TileContext(nc) as tc:
        with (
            tc.tile_pool(name="consts", bufs=1) as consts,
            tc.tile_pool(name="pp", bufs=1) as pp,
            tc.tile_pool(name="sb", bufs=1) as sb,
            tc.tile_pool(name="krg", bufs=3) as krg,
            tc.tile_pool(name="vrg", bufs=3) as vrg,
            tc.tile_pool(name="kTsb", bufs=3) as kTsb,
            tc.tile_pool(name="expp", bufs=4) as expp,
            tc.tile_pool(name="vnew", bufs=4) as vnew,
            tc.tile_pool(name="drain", bufs=2) as drain,
            tc.tile_pool(name="ps", bufs=3, space=bass.MemorySpace.PSUM) as ps,
            tc.tile_pool(name="psT", bufs=3, space=bass.MemorySpace.PSUM) as psT,
            tc.tile_pool(name="psacc", bufs=2, space=bass.MemorySpace.PSUM) as psacc,
        ):
            ident = consts.tile([128, 128], F32)
            nc.sync.dma_start(ident[:], id_d)
            ones = consts.tile([128, 1], F32)
            nc.sync.dma_start(ones[:], ones_d)
            mask = consts.tile([S, S], F32)
            nc.sync.dma_start(mask[:], mask_d)
            cos_sb = consts.tile([P, D // 2], F32)
            nc.sync.dma_start(cos_sb[:], cos_d)
            sin_sb = consts.tile([P, D // 2], F32)
            nc.sync.dma_start(sin_sb[:], sin_d)
            wq_sb = consts.tile([P, D], F32)
            nc.sync.dma_start(wq_sb[:], wq_d)
            wk_sb = consts.tile([P, D], F32)
            nc.sync.dma_start(wk_sb[:], wk_d)
            eps_sb = consts.tile([P, 1], F32)
            nc.vector.memset(eps_sb[:], EPS)

            qT = _preprocess(nc, sb, pp, psT, q_d, wq_sb, cos_sb, sin_sb,
                             ident, eps_sb, "q", dbg_x, dbg_xr)
            kTn = _preprocess(nc, sb, pp, psT, k_d, wk_sb, cos_sb, sin_sb,
                              ident, eps_sb, "k")
            if DEBUG:
                nc.sync.dma_start(dbg_qT[:], qT[:])
                nc.sync.dma_start(dbg_kTn[:], kTn[:])

            for b in range(B_LOC):
                # one PSUM bank for everything this batch accumulates:
                # cols h*8..h*8+4 = oT[d, q] of head h; [0:1, 128+h*8..+4] =
                # sum_j exp of head h. Single accumulation group.
                acc = psacc.tile([128, 256], F32, tag="acc")

                for t in range(TILES):
                    k_rg = krg.tile([128, H * D], F32, tag="k_rg")
                    nc.sync.dma_start(
                        k_rg[:].rearrange("p (h d) -> p h d", h=H),
                        ck_d[b, t * 128:(t + 1) * 128],
                    )
                    v_rg = vrg.tile([128, H * D], F32, tag="v_rg")
                    nc.sync.dma_start(
                        v_rg[:].rearrange("p (h d) -> p h d", h=H),
                        cv_d[b, t * 128:(t + 1) * 128],
                    )
                    for h in range(H):
                        hs = slice(h * D, (h + 1) * D)
                        kT_ps = psT.tile([128, 128], F32, tag="kT_ps")
                        nc.tensor.transpose(kT_ps[:], k_rg[:, hs], ident[:])
                        kT = kTsb.tile([128, 128], F32, tag="kT")
                        nc.vector.tensor_copy(kT[:], kT_ps[:])

                        c = _col(b, h)
                        sT_ps = ps.tile([128, S], F32, tag="sT")
                        nc.tensor.matmul(sT_ps[:], kT[:], qT[:, c:c + S],
                                         start=True, stop=True)
                        expT = expp.tile([128, S], F32, tag="expT")
                        nc.scalar.activation(expT[:], sT_ps[:], AF.Exp,
                                             scale=SCALE)

                        first = (t == 0 and h == 0)
                        nc.tensor.matmul(acc[:, h * 8:h * 8 + S], v_rg[:, hs],
                                         expT[:], start=first, stop=False,
                                         skip_group_check=True)
                        nc.tensor.matmul(acc[0:1, 128 + h * 8:128 + h * 8 + S],
                                         ones[:], expT[:], start=False,
                                         stop=False, skip_group_check=True)

                # the 4 new (current) keys, causal-masked
                for h in range(H):
                    c = _col(b, h)
                    sn_ps = ps.tile([128, S], F32, tag="sT")
                    nc.tensor.matmul(sn_ps[0:S, :], kTn[:, c:c + S],
                                     qT[:, c:c + S], start=True, stop=True)
                    en = expp.tile([128, S], F32, tag="expT")
                    nc.scalar.activation(en[0:S, :], sn_ps[0:S, :], AF.Exp,
                                         scale=SCALE)
                    enm = expp.tile([S, S], F32, tag="enm")
                    nc.vector.tensor_mul(enm[:], en[0:S, :], mask[:])
                    if DEBUG and b == 0:
                        nc.sync.dma_start(dbg_enm[h], enm[:])

                    v_n = vnew.tile([S, D], F32, tag="v_n")
                    nc.sync.dma_start(v_n[:], v_d[b, :, h * D:(h + 1) * D])
                    nc.tensor.matmul(acc[:, h * 8:h * 8 + S], v_n[:], enm[:],
                                     start=False, stop=False,
                                     skip_group_check=True)
                    nc.tensor.matmul(acc[0:1, 128 + h * 8:128 + h * 8 + S],
                                     ones[0:S, :], enm[:], start=False,
                                     stop=(h == H - 1), skip_group_check=True)

                # drain: transpose, normalize, store
                acc_sb = drain.tile([128, 256], F32, tag="acc_sb")
                nc.vector.tensor_copy(acc_sb[:], acc[:])
                if DEBUG and b == 0:
                    nc.sync.dma_start(dbg_acc[:], acc_sb[:])
                o_ps = psT.tile([128, 128], F32, tag="kT_ps")
                nc.tensor.transpose(o_ps[:], acc_sb[:, 0:128], ident[:])
                sums_ps = ps.tile([128, S], F32, tag="sT")
                nc.tensor.transpose(sums_ps[:, 0:1], acc_sb[0:1, 128:256],
                                    ident[0:1, 0:1])
                rs = drain.tile([128, 1], F32, tag="rs")
                nc.vector.reciprocal(rs[:], sums_ps[:, 0:1])
                o_norm = drain.tile([128, 128], F32, tag="o_norm")
                nc.vector.tensor_scalar_mul(o_norm[:], o_ps[:], rs[:])
                for h in range(H):
                    nc.sync.dma_start(
                        out_d[b, :, h * D:(h + 1) * D],
                        o_norm[h * 8:h * 8 + S, :],
                    )

    nc.compile()
    return nc


_NC_CACHE = []


def _get_nc():
    if not _NC_CACHE:
        _NC_CACHE.append(build())
    return _NC_CACHE[0]


def make_in_maps(inputs):
    return _make_in_maps(**inputs)


def _make_in_maps(q, k, v, freqs_cos, freqs_sin, cache_k, cache_v, q_norm_w,
                  k_norm_w):
    q = np.asarray(q, dtype=np.float32)
    k = np.asarray(k, dtype=np.float32)
    v = np.asarray(v, dtype=np.float32)
    cache_k = np.asarray(cache_k, dtype=np.float32)
    cache_v = np.asarray(cache_v, dtype=np.float32)
    freqs_cos = np.asarray(freqs_cos, dtype=np.float32)
    freqs_sin = np.asarray(freqs_sin, dtype=np.float32)
    q_norm_w = np.asarray(q_norm_w, dtype=np.float32)
    k_norm_w = np.asarray(k_norm_w, dtype=np.float32)

    # host-side constant marshalling (layout helpers only)
    cos_b = np.ascontiguousarray(
        np.broadcast_to(freqs_cos[None, None], (B_LOC, H, S, D // 2))
        .reshape(P, D // 2))
    sin_b = np.ascontiguousarray(
        np.broadcast_to(freqs_sin[None, None], (B_LOC, H, S, D // 2))
        .reshape(P, D // 2))
    wq_b = np.ascontiguousarray(np.broadcast_to(q_norm_w[None, :], (P, D)))
    wk_b = np.ascontiguousarray(np.broadcast_to(k_norm_w[None, :], (P, D)))
    ident = np.eye(128, dtype=np.float32)
    ones = np.ones((128, 1), dtype=np.float32)
    # mask[t, i] = 1 if query i attends new key t (i >= t)
    mask = np.triu(np.ones((S, S), dtype=np.float32)).T.copy()
    mask = np.ascontiguousarray(
        (np.arange(S)[None, :] >= np.arange(S)[:, None]).astype(np.float32))

    in_maps = []
    for i in range(N_CORES):
        bs = slice(i * B_LOC, (i + 1) * B_LOC)
        in_maps.append({
            "q": np.ascontiguousarray(q[bs]),
            "k": np.ascontiguousarray(k[bs]),
            "v": np.ascontiguousarray(v[bs]),
            "cache_k": np.ascontiguousarray(cache_k[bs]),
            "cache_v": np.ascontiguousarray(cache_v[bs]),
            "cos_b": cos_b, "sin_b": sin_b, "wq_b": wq_b, "wk_b": wk_b,
            "ident": ident, "ones": ones, "mask": mask,
        })
    return in_maps


def run(q, k, v, freqs_cos, freqs_sin, cache_k, cache_v, q_norm_w, k_norm_w,
        trace=False):
    in_maps = _make_in_maps(q, k, v, freqs_cos, freqs_sin, cache_k, cache_v,
                            q_norm_w, k_norm_w)
    nc = _get_nc()
    res = run_bass_kernel_spmd(nc, in_maps, list(range(N_CORES)), trace=trace)
    out = np.concatenate([res.results[i]["out"] for i in range(N_CORES)],
                         axis=0)
    return out.reshape(B, S, DIM), res


def kernel(q, k, v, freqs_cos, freqs_sin, cache_k, cache_v, q_norm_w,
           k_norm_w):
    out, _ = run(q, k, v, freqs_cos, freqs_sin, cache_k, cache_v, q_norm_w,
                 k_norm_w)
    return out
